# revision 7
# baseline (speedup 1.0000x reference)
"""Trainium2 Bass kernel for nn_KGather (sparse_attention gather+scale).

Reference computation:
    out[n, p, t, w, c] = r_weight[n, p, t] * k[n, r_idx[n, p, t], w, c]
with n=16, p2=49, topk=8, w2=64, ck=128 (all fp32; r_idx int).

Strategy (8 cores, data parallel over n, 2 batch elements per core):
  - Host side: fold the gather indices AND the routing weights into a
    block-diagonal scaled one-hot matrix per core:
        onehot[j, pt] = r_weight[n_l, p, t]  if j == n_l*49 + r_idx[n_l, p, t]
    with pt = (n_l*49 + p)*8 + t, j in [0, 98).
  - Device side (static program, data-independent):
        out_core[pt, wc] = sum_j onehot[j, pt] * k_core[j, wc]
    i.e. a dense matmul on the TensorEngine. Everything on device runs in
    bf16 (the problem tolerates rel err 2e-2; bf16 contributes ~4e-3):
    the one-hot+k input is bf16 (1.76 MB/core read once) and the output is
    written to HBM as bf16 (12.8 MB/core) then upcast to fp32 on the host.
    This halves the dominant HBM write traffic vs fp32 and runs the
    matmul at full bf16 rate (fp32 matmul is ~4x slower and was the
    previous bottleneck).
  - PSUM tiles of [112, 1024] fp32 (2 banks) are filled by two 512-wide
    matmuls and drained to bf16 SBUF staging by ONE copy instruction each;
    drains are split across ScalarE and VectorE (~55/45) so neither
    engine exceeds the DMA store time. Output leaves in 7 contiguous
    1.84 MB DMA stores (8 DMAs total -> no HWDGE-lane-reuse waits).

Each one-hot column has exactly one nonzero, so the matmul reproduces
r_weight * k exactly up to bf16 rounding of w, k, and the product.
"""

import numpy as np
import ml_dtypes

BF16 = ml_dtypes.bfloat16

# Problem shape (hardcoded per contest rules).
N, P2, TOPK, W2, CK = 16, 49, 8, 64, 128
NCORES = 8
NB = N // NCORES          # batch elements per core = 2
ROWS = NB * P2            # contraction dim per core = 98
PT = NB * P2 * TOPK       # output windows per core = 784
WC = W2 * CK              # window elements = 8192
PT_CHUNK = 112            # 7 pt chunks of 112 (<=128 partitions)
MM_CHUNK = 512            # matmul moving free size (this walrus build
                          # rejects 1024-wide Matmult: "ISA check failed")
DR_CHUNK = 2048           # drain free size (one [112,2048] PSUM tile)
ST_CHUNK = 4096           # store free size (two drains per half-store)
LD_SPLIT = 4096           # k columns in the first (serialized) load

_PROGRAM_CACHE = {}


def _drain_engine_schedule():
    """Assign each of the 56 drains to ScalarE (ACT) or VectorE (DVE).

    Cost per [112,1024] drain: ACT (1024+172)/1.2GHz = 997ns,
    DVE (1024+120)/0.96GHz = 1192ns.  Balanced split: ACT share
    f = 1192/(997+1192) = 0.545 -> 30 ACT / 26 DVE, interleaved.
    """
    n = (PT // PT_CHUNK) * (WC // DR_CHUNK)
    # ACT (2048+172)/1.2GHz = 1850ns, DVE (2048+120)/0.96GHz = 2258ns.
    n_act = round(n * 2258.0 / (1850.0 + 2258.0))
    sched = []
    acc = 0
    for j in range(n):
        nxt = (j + 1) * n_act // n
        sched.append("act" if nxt > acc else "dve")
        acc = nxt
    return sched


def _build_program(patch=True):
    """Build the (data-independent) per-core Bass program.

    patch=True applies _split_multi_waits (required for the HW compile;
    the JSON round-trip breaks CoreSim, so use patch=False for sim)."""
    import concourse.bass as bass
    import concourse.mybir as mybir
    import concourse.tile as tile

    nc = bass.Bass()
    # onehot and k_core are packed into one input ([98, 784+8192]) so the
    # whole load is ONE DMA -> one completion semaphore.
    koh_d = nc.dram_tensor("koh", [ROWS, PT + WC], mybir.dt.bfloat16,
                           kind="ExternalInput")
    out_d = nc.dram_tensor("out_core", [PT, WC], mybir.dt.bfloat16,
                           kind="ExternalOutput")

    bf16 = mybir.dt.bfloat16
    f32 = mybir.dt.float32
    n_cp = PT // PT_CHUNK
    n_st = WC // ST_CHUNK
    dr_per_st = ST_CHUNK // DR_CHUNK
    mm_per_dr = DR_CHUNK // MM_CHUNK
    sched = _drain_engine_schedule()

    with tile.TileContext(nc) as tc:
        with (
            tc.tile_pool(name="const", bufs=1) as cpool,
            tc.tile_pool(name="stage", bufs=3) as spool,
            tc.tile_pool(name="psum", bufs=2, space="PSUM") as ppool,
        ):
            koh_sb = cpool.tile([ROWS, PT + WC], bf16)
            # PE warm-up: HAM unthrottles the PE clock (1.2 -> 2.4 GHz)
            # only after ~3.4us of sustained activity.  Matmul garbage
            # into the psum pool while the load DMA streams, so the real
            # matmuls start warm.  Inputs are memzeroed to keep birsim
            # happy; psum pool slots are recycled by the real matmuls.
            wu_lhsT = cpool.tile([ROWS, PT_CHUNK], bf16)
            wu_rhs = cpool.tile([ROWS, MM_CHUNK], bf16)
            nc.scalar.memzero(wu_lhsT[:])
            nc.scalar.memzero(wu_rhs[:])
            for _ in range(7):
                wu_ps = ppool.tile([PT_CHUNK, DR_CHUNK], f32, space="PSUM")
                nc.tensor.matmul(wu_ps[:, :MM_CHUNK], lhsT=wu_lhsT[:],
                                 rhs=wu_rhs[:], start=True, stop=True)
            # Two loads so matmuls can start after the first ~1 MB: the
            # one-column overlap (column PT+LD_SPLIT, which only the
            # second-half matmuls read) is a deliberate WAW dep that makes
            # Tile serialize load-b after load-a (concurrent DMAs would
            # share SDMA bandwidth round-robin and delay load-a instead).
            nc.sync.dma_start(out=koh_sb[:, :PT + LD_SPLIT + 1],
                              in_=koh_d[:, :PT + LD_SPLIT + 1])
            nc.sync.dma_start(out=koh_sb[:, PT + LD_SPLIT:],
                              in_=koh_d[:, PT + LD_SPLIT:])

            di = 0
            for cp in range(n_cp):
                stage = spool.tile([PT_CHUNK, WC], bf16)
                lhsT = koh_sb[:, cp * PT_CHUNK:(cp + 1) * PT_CHUNK]
                for st in range(n_st):
                    for dr in range(dr_per_st):
                        ps = ppool.tile([PT_CHUNK, DR_CHUNK], f32,
                                        space="PSUM")
                        base = st * ST_CHUNK + dr * DR_CHUNK
                        for m in range(mm_per_dr):
                            rhs = koh_sb[:, PT + base + m * MM_CHUNK:
                                         PT + base + (m + 1) * MM_CHUNK]
                            nc.tensor.matmul(
                                ps[:, m * MM_CHUNK:(m + 1) * MM_CHUNK],
                                lhsT=lhsT, rhs=rhs, start=True, stop=True)
                        sl = slice(base, base + DR_CHUNK)
                        if sched[di] == "act":
                            nc.scalar.copy(out=stage[:, sl], in_=ps[:])
                        else:
                            nc.vector.tensor_copy(out=stage[:, sl], in_=ps[:])
                        di += 1
                    rows = slice(cp * PT_CHUNK, (cp + 1) * PT_CHUNK)
                    cols = slice(st * ST_CHUNK, (st + 1) * ST_CHUNK)
                    nc.sync.dma_start(out=out_d[rows, cols],
                                      in_=stage[:, cols])
    if patch:
        _split_multi_waits(nc)
    return nc


def _split_multi_waits(nc):
    """This walrus build rejects >1 fused sync-wait per instruction
    ("Too many sync wait commands"). Tile's wait assigner happily fuses
    several. Rewrite the BIR: for any instruction with N>1 waits, emit
    N-1 standalone single-wait EventSemaphore instructions (same engine,
    immediately before it) and keep only the last wait fused."""
    import json
    from concourse import mybir

    j = json.loads(mybir.module_to_json_string(nc.m))
    uid = [0]
    for f in j["functions"]:
        for b in f["blocks"]:
            out = []
            for ins in b["instructions"]:
                sync = ins.get("sync_info") or {}
                waits = sync.get("on_wait") or []
                if len(waits) > 1:
                    for w in waits[:-1]:
                        uid[0] += 1
                        out.append({
                            "debug": ins.get("debug", 0),
                            "engine": ins["engine"],
                            "ins": [],
                            "name": f"wsplit-{uid[0]}-{ins['name']}",
                            "opcode": "EventSemaphore",
                            "outs": [],
                            "sync_info": {"on_update": [], "on_wait": [w]},
                        })
                    sync["on_wait"] = [waits[-1]]
                out.append(ins)
            b["instructions"] = out
    nc.m = mybir.parse(j)


def get_program():
    if "nc" not in _PROGRAM_CACHE:
        _PROGRAM_CACHE["nc"] = _build_program()
    return _PROGRAM_CACHE["nc"]


def build_in_maps(r_idx, r_weight, k):
    """Host-side sharding + preprocessing: per-core inputs for the program."""
    r_idx = np.asarray(r_idx).astype(np.int64)
    r_weight = np.asarray(r_weight).astype(BF16)
    k = np.asarray(k).astype(BF16)

    pt = np.arange(PT)
    n_l = pt // (P2 * TOPK)
    p = (pt // TOPK) % P2
    t = pt % TOPK

    in_maps = []
    for c in range(NCORES):
        n0 = c * NB
        idx = r_idx[n0:n0 + NB]
        wgt = r_weight[n0:n0 + NB]
        koh = np.empty((ROWS, PT + WC), BF16)
        koh[:, :PT] = BF16(0.0)
        rows = n_l * P2 + idx[n_l, p, t]
        koh[rows, pt] = wgt[n_l, p, t]
        koh[:, PT:] = k[n0:n0 + NB].reshape(ROWS, WC)
        in_maps.append({"koh": koh})
    return in_maps


def run_program(in_maps, trace=False, **kwargs):
    from concourse.bass_utils import run_bass_kernel_spmd
    return run_bass_kernel_spmd(get_program(), in_maps,
                                list(range(NCORES)), trace=trace, **kwargs)


def assemble_output(results):
    out = np.empty((N, P2, TOPK, W2, CK), np.float32)
    for c in range(NCORES):
        out[c * NB:(c + 1) * NB] = np.asarray(
            results[c]["out_core"]).astype(np.float32).reshape(
            NB, P2, TOPK, W2, CK)
    return out


def kernel(r_idx, r_weight, k):
    in_maps = build_in_maps(r_idx, r_weight, k)
    res = run_program(in_maps)
    return assemble_output(res.results)


# revision 10
# speedup vs baseline: 1.3069x; 1.3069x over previous
"""Trainium2 Bass kernel for nn_KGather (sparse_attention gather+scale).

Reference computation:
    out[n, p, t, w, c] = r_weight[n, p, t] * k[n, r_idx[n, p, t], w, c]
with n=16, p2=49, topk=8, w2=64, ck=128 (all fp32; r_idx int).

Strategy (8 cores, data parallel over n, 2 batch elements per core):
  - Host side: fold the gather indices AND the routing weights into a
    block-diagonal scaled one-hot matrix per core:
        onehot[j, pt] = r_weight[n_l, p, t]  if j == n_l*49 + r_idx[n_l, p, t]
    with pt = (n_l*49 + p)*8 + t, j in [0, 98).
  - Device side (static program, data-independent):
        out_core[pt, wc] = sum_j onehot[j, pt] * k_core[j, wc]
    i.e. a dense matmul on the TensorEngine. Everything on device runs in
    bf16 (the problem tolerates rel err 2e-2; bf16 contributes ~4e-3):
    the one-hot+k input is bf16 (1.76 MB/core read once) and the output is
    written to HBM as bf16 (12.8 MB/core) then upcast to fp32 on the host.
    This halves the dominant HBM write traffic vs fp32 and runs the
    matmul at full bf16 rate (fp32 matmul is ~4x slower and was the
    previous bottleneck).
  - PSUM tiles of [112, 1024] fp32 (2 banks) are filled by two 512-wide
    matmuls and drained to bf16 SBUF staging by ONE copy instruction each;
    drains are split across ScalarE and VectorE (~55/45) so neither
    engine exceeds the DMA store time. Output leaves in 7 contiguous
    1.84 MB DMA stores (8 DMAs total -> no HWDGE-lane-reuse waits).

Each one-hot column has exactly one nonzero, so the matmul reproduces
r_weight * k exactly up to bf16 rounding of w, k, and the product.
"""

import numpy as np
import ml_dtypes

BF16 = ml_dtypes.bfloat16

# Problem shape (hardcoded per contest rules).
N, P2, TOPK, W2, CK = 16, 49, 8, 64, 128
NCORES = 8
NB = N // NCORES          # batch elements per core = 2
ROWS = NB * P2            # contraction dim per core = 98
PT = NB * P2 * TOPK       # output windows per core = 784
WC = W2 * CK              # window elements = 8192
PT_CHUNK = 112            # 7 pt chunks of 112 (<=128 partitions)
MM_CHUNK = 512            # matmul moving free size (this walrus build
                          # rejects 1024-wide Matmult: "ISA check failed")
DR_CHUNK = 1024           # drain free size (one [112,1024] PSUM tile;
                          # 2048-wide drains with only 2 PSUM slots stall
                          # the PE long enough for HAM to re-throttle it)
ST_CHUNK = 4096           # store free size (two drains per half-store)
LD_SPLIT = 4096           # k columns in the first (serialized) load

_PROGRAM_CACHE = {}


def _drain_engine_schedule():
    """Assign each of the 56 drains to ScalarE (ACT) or VectorE (DVE).

    Cost per [112,1024] drain: ACT (1024+172)/1.2GHz = 997ns,
    DVE (1024+120)/0.96GHz = 1192ns.  Balanced split: ACT share
    f = 1192/(997+1192) = 0.545 -> 30 ACT / 26 DVE, interleaved.
    """
    n = (PT // PT_CHUNK) * (WC // DR_CHUNK)
    # ACT (1024+172)/1.2GHz = 997ns, DVE (1024+120)/0.96GHz = 1192ns.
    n_act = round(n * 1192.0 / (997.0 + 1192.0))
    sched = []
    acc = 0
    for j in range(n):
        nxt = (j + 1) * n_act // n
        sched.append("act" if nxt > acc else "dve")
        acc = nxt
    return sched


def _build_program(patch=True):
    """Build the (data-independent) per-core Bass program.

    patch=True applies _split_multi_waits (required for the HW compile;
    the JSON round-trip breaks CoreSim, so use patch=False for sim)."""
    import concourse.bass as bass
    import concourse.mybir as mybir
    import concourse.tile as tile

    nc = bass.Bass()
    # onehot and k_core are packed into one input ([98, 784+8192]) so the
    # whole load is ONE DMA -> one completion semaphore.
    koh_d = nc.dram_tensor("koh", [ROWS, PT + WC], mybir.dt.bfloat16,
                           kind="ExternalInput")
    out_d = nc.dram_tensor("out_core", [PT, WC], mybir.dt.bfloat16,
                           kind="ExternalOutput")

    bf16 = mybir.dt.bfloat16
    f32 = mybir.dt.float32
    n_cp = PT // PT_CHUNK
    n_st = WC // ST_CHUNK
    dr_per_st = ST_CHUNK // DR_CHUNK
    mm_per_dr = DR_CHUNK // MM_CHUNK
    sched = _drain_engine_schedule()

    with tile.TileContext(nc) as tc:
        with (
            tc.tile_pool(name="const", bufs=1) as cpool,
            tc.tile_pool(name="stage", bufs=3) as spool,
            tc.tile_pool(name="psum", bufs=4, space="PSUM") as ppool,
        ):
            koh_sb = cpool.tile([ROWS, PT + WC], bf16)
            # PE warm-up: HAM unthrottles the PE clock (1.2 -> 2.4 GHz)
            # only after ~3.4us of sustained activity.  Matmul garbage
            # into the psum pool while the load DMA streams, so the real
            # matmuls start warm.  Inputs are memzeroed to keep birsim
            # happy; psum pool slots are recycled by the real matmuls.
            wu_lhsT = cpool.tile([ROWS, PT_CHUNK], bf16)
            wu_rhs = cpool.tile([ROWS, MM_CHUNK], bf16)
            wu_out = cpool.tile([1, 1], f32)
            nc.scalar.memzero(wu_lhsT[:])
            nc.scalar.memzero(wu_rhs[:])
            wu_ps = ppool.tile([PT_CHUNK, DR_CHUNK], f32, space="PSUM",
                               tag="ps")
            for _ in range(7):
                nc.tensor.matmul(wu_ps[:, :MM_CHUNK], lhsT=wu_lhsT[:],
                                 rhs=wu_rhs[:], start=True, stop=True)
            # one tiny read so the pool slot is freed for the main loop
            nc.vector.tensor_copy(out=wu_out[:], in_=wu_ps[:1, :1])
            # Two loads so matmuls can start after the first ~1 MB: the
            # one-column overlap (column PT+LD_SPLIT, which only the
            # second-half matmuls read) is a deliberate WAW dep that makes
            # Tile serialize load-b after load-a (concurrent DMAs would
            # share SDMA bandwidth round-robin and delay load-a instead).
            nc.sync.dma_start(out=koh_sb[:, :PT + LD_SPLIT + 1],
                              in_=koh_d[:, :PT + LD_SPLIT + 1])
            nc.sync.dma_start(out=koh_sb[:, PT + LD_SPLIT:],
                              in_=koh_d[:, PT + LD_SPLIT:])

            di = 0
            for cp in range(n_cp):
                stage = spool.tile([PT_CHUNK, WC], bf16)
                lhsT = koh_sb[:, cp * PT_CHUNK:(cp + 1) * PT_CHUNK]
                for st in range(n_st):
                    for dr in range(dr_per_st):
                        ps = ppool.tile([PT_CHUNK, DR_CHUNK], f32,
                                        space="PSUM", tag="ps")
                        base = st * ST_CHUNK + dr * DR_CHUNK
                        for m in range(mm_per_dr):
                            rhs = koh_sb[:, PT + base + m * MM_CHUNK:
                                         PT + base + (m + 1) * MM_CHUNK]
                            nc.tensor.matmul(
                                ps[:, m * MM_CHUNK:(m + 1) * MM_CHUNK],
                                lhsT=lhsT, rhs=rhs, start=True, stop=True)
                        sl = slice(base, base + DR_CHUNK)
                        if sched[di] == "act":
                            nc.scalar.copy(out=stage[:, sl], in_=ps[:])
                        else:
                            nc.vector.tensor_copy(out=stage[:, sl], in_=ps[:])
                        di += 1
                    rows = slice(cp * PT_CHUNK, (cp + 1) * PT_CHUNK)
                    cols = slice(st * ST_CHUNK, (st + 1) * ST_CHUNK)
                    nc.sync.dma_start(out=out_d[rows, cols],
                                      in_=stage[:, cols])
    if patch:
        _split_multi_waits(nc)
    return nc


def _split_multi_waits(nc):
    """This walrus build rejects >1 fused sync-wait per instruction
    ("Too many sync wait commands"). Tile's wait assigner happily fuses
    several. Rewrite the BIR: for any instruction with N>1 waits, emit
    N-1 standalone single-wait EventSemaphore instructions (same engine,
    immediately before it) and keep only the last wait fused."""
    import json
    from concourse import mybir

    j = json.loads(mybir.module_to_json_string(nc.m))
    uid = [0]
    for f in j["functions"]:
        for b in f["blocks"]:
            out = []
            for ins in b["instructions"]:
                sync = ins.get("sync_info") or {}
                waits = sync.get("on_wait") or []
                if len(waits) > 1:
                    for w in waits[:-1]:
                        uid[0] += 1
                        out.append({
                            "debug": ins.get("debug", 0),
                            "engine": ins["engine"],
                            "ins": [],
                            "name": f"wsplit-{uid[0]}-{ins['name']}",
                            "opcode": "EventSemaphore",
                            "outs": [],
                            "sync_info": {"on_update": [], "on_wait": [w]},
                        })
                    sync["on_wait"] = [waits[-1]]
                out.append(ins)
            b["instructions"] = out
    nc.m = mybir.parse(j)


def get_program():
    if "nc" not in _PROGRAM_CACHE:
        _PROGRAM_CACHE["nc"] = _build_program()
    return _PROGRAM_CACHE["nc"]


def build_in_maps(r_idx, r_weight, k):
    """Host-side sharding + preprocessing: per-core inputs for the program."""
    r_idx = np.asarray(r_idx).astype(np.int64)
    r_weight = np.asarray(r_weight).astype(BF16)
    k = np.asarray(k).astype(BF16)

    pt = np.arange(PT)
    n_l = pt // (P2 * TOPK)
    p = (pt // TOPK) % P2
    t = pt % TOPK

    in_maps = []
    for c in range(NCORES):
        n0 = c * NB
        idx = r_idx[n0:n0 + NB]
        wgt = r_weight[n0:n0 + NB]
        koh = np.empty((ROWS, PT + WC), BF16)
        koh[:, :PT] = BF16(0.0)
        rows = n_l * P2 + idx[n_l, p, t]
        koh[rows, pt] = wgt[n_l, p, t]
        koh[:, PT:] = k[n0:n0 + NB].reshape(ROWS, WC)
        in_maps.append({"koh": koh})
    return in_maps


def run_program(in_maps, trace=False, **kwargs):
    from concourse.bass_utils import run_bass_kernel_spmd
    return run_bass_kernel_spmd(get_program(), in_maps,
                                list(range(NCORES)), trace=trace, **kwargs)


def assemble_output(results):
    out = np.empty((N, P2, TOPK, W2, CK), np.float32)
    for c in range(NCORES):
        out[c * NB:(c + 1) * NB] = np.asarray(
            results[c]["out_core"]).astype(np.float32).reshape(
            NB, P2, TOPK, W2, CK)
    return out


def kernel(r_idx, r_weight, k):
    in_maps = build_in_maps(r_idx, r_weight, k)
    res = run_program(in_maps)
    return assemble_output(res.results)
